# revision 24
# baseline (speedup 1.0000x reference)
"""Trainium2 Bass kernel for the attention-style channel-mixing block
(3x 1x1-conv + double softmax feature gathering + projection + BN).

Data-parallel over batch: B=16 split as 2 batch items per NeuronCore
across 8 cores; weights replicated. No cross-core communication.

Math per batch item (xf = x[b] reshaped [C=512, N=4096]):
    ZA = wA @ xf ; ZB = wB @ xf ; ZC = wC @ xf          (Hd=128 rows)
    A  = softmax(ZA, axis=n) ; Cs = softmax(ZC, axis=n)
    G  = ZB @ A^T                                        [128,128]
    y  = G @ Cs
    out = BN(wProj @ y)

Softmax denominators are folded into the tiny G matrix:
    y = (G * (1/(sumA*sumC))[None,:]) @ expC
so the only full-size nonlinear work is exp() on the scalar engine.
No max-subtraction is needed: |Z| < ~6 so exp() stays in fp32 range.

ZA/ZB are computed directly in transposed layout (n on partitions) so
G needs no separate TensorE transposes; sumA falls out of a
matmul-with-ones that shares G's PSUM accumulation group (single
start=True — a second group start would clear the whole bank's
has_written bits and drop G's first tile).

The two batch items are software-pipelined: phase 2 of batch 0 (y,
projection, store — little PE work) is interleaved instruction-by-
instruction with phase 1 of batch 1 (matmul-dense) so the in-order
TensorE queue never drains.

Compute dtype is bf16 (inputs cast on host, fp32 PSUM accumulation),
output fp32. Measured end-to-end rel err vs the fp32 reference ~4.7e-3.
"""

import contextlib
import sys

if "/opt/trn_rl_repo" not in sys.path:
    sys.path.insert(0, "/opt/trn_rl_repo")

import ml_dtypes
import numpy as np

import concourse.bass as bass
import concourse.mybir as mybir
import concourse.tile as tile
from concourse import bacc
from concourse.bass_utils import run_bass_kernel_spmd
from concourse.masks import make_identity

BF16 = mybir.dt.bfloat16
F32 = mybir.dt.float32

B, C, HD = 16, 512, 128
N = 64 * 64
NCORES = 8
BPC = B // NCORES          # batch items per core
KC = C // 128              # 4 contraction tiles over channels
MC = C // 128              # 4 output-row tiles of the projection
CH = 512                   # free-dim chunk (one PSUM bank of fp32)
NCH = N // CH              # 8 chunks
TPC = CH // 128            # 4 transposed 128-tiles per chunk
XDMA = 4                   # input DMAs per batch item

_CACHE: dict = {}


def ts(i, size):
    return slice(i * size, (i + 1) * size)


def build_nc(defer_g=True, warmup=32, stagger=2, big_bufs=4, p_bufs=3,
             n_act_base=1, fuse_ab=False, z_bufs=3, ab_bufs=2,
             suma_row=False, pre_emit=2, tail2=0, och_bufs=6,
             tail_yfirst=0, ych_bufs=6, spread=0, early_dma=0):
    nc = bacc.Bacc("TRN2", target_bir_lowering=False, debug=False,
                   num_devices=NCORES)

    xs = nc.declare_dram_parameter("xs", [BPC, C, N], BF16, isOutput=False)
    # All three conv weights are staged as w.T split into K-tiles:
    # [p, kc, h] = w[h, kc*128+p]
    if fuse_ab:
        wab = nc.declare_dram_parameter("wab", [128, KC, 2 * HD], BF16,
                                        isOutput=False)
    else:
        wat = nc.declare_dram_parameter("wat", [128, KC, HD], BF16,
                                        isOutput=False)
        wbt = nc.declare_dram_parameter("wbt", [128, KC, HD], BF16,
                                        isOutput=False)
    wct = nc.declare_dram_parameter("wct", [128, KC, HD], BF16, isOutput=False)
    # Projection (BN scale folded) transposed: [p(h), mc, m] = wProj'[mc*128+m, p]
    wpt = nc.declare_dram_parameter("wpt", [128, MC, 128], BF16, isOutput=False)
    biasd = nc.declare_dram_parameter("bias", [128, MC], F32, isOutput=False)
    outd = nc.declare_dram_parameter("out", [BPC, C, N], BF16, isOutput=True)

    with tile.TileContext(nc) as tc:
        with (
            tc.tile_pool(name="const", bufs=1) as cpool,
            tc.tile_pool(name="xin", bufs=2) as xpool,
            tc.tile_pool(name="persist", bufs=2) as ppool,
            tc.tile_pool(name="work", bufs=3) as wpool,
            tc.tile_pool(name="outs", bufs=och_bufs) as opool,
            tc.tile_pool(name="psbig",
                         bufs=(z_bufs if fuse_ab else big_bufs),
                         space="PSUM") as psbig,
            tc.tile_pool(name="psg", bufs=1, space="PSUM") as psg,
            tc.tile_pool(name="psp", bufs=p_bufs, space="PSUM") as psp,
            contextlib.ExitStack() as _ps,
        ):
            psab = (_ps.enter_context(
                tc.tile_pool(name="psab", bufs=ab_bufs, space="PSUM"))
                if fuse_ab else None)
            if warmup:
                warm = cpool.tile([128, 128], BF16, tag="warm")
                nc.gpsimd.memset(warm[:], 0.0)
                psum_warm = psp.tile([128, 128], F32, tag="p", name="warmp")
                for _ in range(warmup):
                    nc.tensor.matmul(psum_warm[:], warm[:], warm[:])

            ident = cpool.tile([128, 128], BF16, tag="ident")
            make_identity(nc, ident[:])
            ones = cpool.tile([128, 1], BF16, tag="ones")
            nc.gpsimd.memset(ones[:], 1.0)
            onesf = cpool.tile([1, 1], F32, tag="onesf")
            nc.gpsimd.memset(onesf[:], 1.0)

            wctt = cpool.tile([128, KC, HD], BF16, tag="wct")
            wptt = cpool.tile([128, MC, 128], BF16, tag="wpt")
            biast = cpool.tile([128, MC], F32, tag="bias")
            nc.scalar.dma_start(wctt[:], wct[:])
            if fuse_ab:
                wabt = cpool.tile([128, KC, 2 * HD], BF16, tag="wab")
                nc.scalar.dma_start(wabt[:], wab[:])
            else:
                watt = cpool.tile([128, KC, HD], BF16, tag="wat")
                wbtt = cpool.tile([128, KC, HD], BF16, tag="wbt")
                nc.scalar.dma_start(watt[:], wat[:])
                nc.scalar.dma_start(wbtt[:], wbt[:])
            nc.scalar.dma_start(wptt[:], wpt[:])
            nc.scalar.dma_start(biast[:], biasd[:])

            # per-batch state, filled in by the emitters below
            st = [dict() for _ in range(BPC)]

            # One PSUM bank holds G+sumA for BOTH batch items ([:, b, :]).
            # Only the literal first G matmul of batch 0 carries start=True;
            # batch 1's first write lands on untouched has_written bits and
            # overwrites per-element, so no second group start is needed
            # (a start would clear the whole bank under batch 0's values).
            gsa_all = psg.tile([128, BPC, 129], F32, tag="g", name="gsa")

            def load_x(b):
                s = st[b]
                s["xt"] = xpool.tile([128, KC, N], BF16, tag="x", name=f"xt{b}")
                xv = xs[b].rearrange("(kc p) n -> p kc n", p=128)
                for h in range(NCH):
                    if b == 0 and h < 2:
                        # land the first chunks fast: four parallel engines
                        for kc in range(KC):
                            nc.sync.dma_start(
                                s["xt"][:, kc:kc + 1, ts(h, CH)],
                                xv[:, kc:kc + 1, ts(h, CH)])
                    else:
                        nc.sync.dma_start(s["xt"][:, :, ts(h, CH)],
                                          xv[:, :, ts(h, CH)])

            def alloc_batch(b):
                s = st[b]
                s["expc"] = ppool.tile([128, N], BF16, tag="expc", name=f"expc{b}")
                s["expat"] = ppool.tile([128, N // 128, 128], BF16, tag="expat", name=f"expat{b}")
                s["bft"] = ppool.tile([128, N // 128, 128], BF16, tag="bft", name=f"bft{b}")
                s["sumcp"] = ppool.tile([128, NCH], F32, tag="sumcp", name=f"sumcp{b}")

            def phase1_chunk(b, j):
                s = st[b]
                xt = s["xt"]

                # ZC = wC @ x  (h on partitions) -> exp + row-sums
                psum_zc = psbig.tile([128, CH], F32, tag="big")
                for kc in range(KC):
                    nc.tensor.matmul(psum_zc[:], wctt[:, kc, :],
                                     xt[:, kc, ts(j, CH)],
                                     start=(kc == 0), stop=(kc == KC - 1))
                nc.scalar.activation(s["expc"][:, ts(j, CH)], psum_zc[:],
                                     mybir.ActivationFunctionType.Exp,
                                     accum_out=s["sumcp"][:, j:j + 1])

                if fuse_ab:
                    # ZA^T and ZB^T in one stream: shared stationary x tile,
                    # rhs = [wA^T | wB^T] (256 cols), halves LDWEIGHTS count
                    for half in range(2):
                        psum_ab = psab.tile([128, 2, 2 * HD], F32, tag="ab",
                                            name=f"ab{b}_{j}_{half}")
                        for tt in range(2):
                            t = 2 * half + tt
                            for kc in range(KC):
                                nc.tensor.matmul(
                                    psum_ab[:, tt, :],
                                    xt[:, kc, ts(j * TPC + t, 128)],
                                    wabt[:, kc, :],
                                    start=(kc == 0), stop=(kc == KC - 1))
                        base = j * TPC + 2 * half
                        nc.scalar.activation(
                            s["expat"][:, base:base + 2, :],
                            psum_ab[:, :, 0:HD],
                            mybir.ActivationFunctionType.Exp)
                        nc.vector.tensor_copy(s["bft"][:, base:base + 2, :],
                                              psum_ab[:, :, HD:2 * HD])
                else:
                    # ZA^T directly (n on partitions): x_tile^T @ wA^T
                    psum_at = psbig.tile([128, TPC, 128], F32, tag="big")
                    for t in range(TPC):
                        for kc in range(KC):
                            nc.tensor.matmul(psum_at[:, t, :],
                                             xt[:, kc, ts(j * TPC + t, 128)],
                                             watt[:, kc, :],
                                             start=(kc == 0),
                                             stop=(kc == KC - 1))
                    nc.scalar.activation(s["expat"][:, ts(j, TPC), :],
                                         psum_at[:],
                                         mybir.ActivationFunctionType.Exp)

                    # ZB^T directly
                    psum_bt = psbig.tile([128, TPC, 128], F32, tag="big")
                    for t in range(TPC):
                        for kc in range(KC):
                            nc.tensor.matmul(psum_bt[:, t, :],
                                             xt[:, kc, ts(j * TPC + t, 128)],
                                             wbtt[:, kc, :],
                                             start=(kc == 0),
                                             stop=(kc == KC - 1))
                    nc.vector.tensor_copy(s["bft"][:, ts(j, TPC), :],
                                          psum_bt[:])

                # G/sumA accumulation for the PREVIOUS chunk: its evacs are
                # long done, so these matmuls never stall the in-order PE
                # queue waiting on DVE/ACT.
                if defer_g:
                    if j > 0:
                        g_accum(b, j - 1)
                else:
                    g_accum(b, j)

            def g_accum(b, j):
                # G += ZB^T.T @ expA^T ; sumA += expA^T.T @ ones
                s = st[b]
                for t in range(TPC):
                    first = (b == 0 and j == 0 and t == 0)
                    last = (b == BPC - 1 and j == NCH - 1 and t == TPC - 1)
                    nc.tensor.matmul(gsa_all[:, b, 0:128],
                                     s["bft"][:, j * TPC + t, :],
                                     s["expat"][:, j * TPC + t, :],
                                     start=first, stop=False,
                                     skip_group_check=True)
                    nc.tensor.matmul(gsa_all[:, b, 128:129],
                                     s["expat"][:, j * TPC + t, :],
                                     ones[:, :1],
                                     start=False, stop=last,
                                     skip_group_check=True)

            def finalize(b):
                s = st[b]
                if defer_g:
                    g_accum(b, NCH - 1)
                sumc = wpool.tile([128, 1], F32, tag="sumc")
                nc.vector.tensor_reduce(sumc[:], s["sumcp"][:],
                                        mybir.AxisListType.X,
                                        mybir.AluOpType.add)
                prod = wpool.tile([128, 1], F32, tag="prod")
                nc.vector.tensor_scalar_mul(prod[:], gsa_all[:, b, 128:129],
                                            sumc[:])
                sinv = wpool.tile([128, 1], F32, tag="sinv")
                nc.vector.reciprocal(sinv[:], prod[:])

                g_sb = wpool.tile([128, 128], BF16, tag="g")
                nc.vector.tensor_copy(g_sb[:], gsa_all[:, b, 0:128])
                psum_gt = psp.tile([128, 128], BF16, tag="p")
                nc.tensor.transpose(psum_gt[:], g_sb[:], ident[:])
                s["gt"] = wpool.tile([128, 128], BF16, tag="gt", name=f"gt{b}")
                nc.scalar.activation(s["gt"][:], psum_gt[:],
                                     mybir.ActivationFunctionType.Copy,
                                     scale=sinv[:, :1])

            def phase2_y(b, j):
                s = st[b]
                psum_y = psbig.tile([128, CH], F32, tag="big")
                nc.tensor.matmul(psum_y[:], s["gt"][:], s["expc"][:, ts(j, CH)])
                s[f"ych{j}"] = wpool.tile([128, CH], BF16, tag="ych", name=f"ych{b}_{j}", bufs=ych_bufs)
                nc.scalar.activation(s[f"ych{j}"][:], psum_y[:],
                                     mybir.ActivationFunctionType.Copy)

            def phase2_p(b, j):
                s = st[b]
                ov = outd[b].rearrange("(mc p) n -> p mc n", p=128)
                och = opool.tile([128, MC, CH], BF16, tag="och", name=f"och{b}_{j}")
                n_act = n_act_base + (j % 2)  # avg n_act_base+0.5 evacs on ACT
                for mc in range(MC):
                    psum_p = psp.tile([128, CH], F32, tag="p")
                    nc.tensor.matmul(psum_p[:], wptt[:, mc, :], s[f"ych{j}"][:])
                    dst = och[:, mc, :]
                    if mc < n_act:
                        nc.scalar.activation(
                            dst, psum_p[:],
                            mybir.ActivationFunctionType.Identity,
                            bias=biast[:, mc:mc + 1])
                    else:
                        nc.vector.tensor_scalar_add(dst, psum_p[:],
                                                    biast[:, mc:mc + 1])
                    if early_dma and mc == 1 and not (
                            b == BPC - 1 and j >= NCH - 2):
                        # ship the first half as soon as its rows are ready
                        nc.sync.dma_start(ov[:, 0:2, ts(j, CH)],
                                          och[:, 0:2, :])
                if early_dma and not (b == BPC - 1 and j >= NCH - 2):
                    nc.sync.dma_start(ov[:, 2:4, ts(j, CH)], och[:, 2:4, :])
                elif b == BPC - 1 and j >= NCH - 2:
                    # end-of-kernel drain: max DMA-engine parallelism
                    for mc in range(MC):
                        nc.sync.dma_start(ov[:, mc:mc + 1, ts(j, CH)],
                                          och[:, mc:mc + 1, :])
                elif not early_dma:
                    nc.sync.dma_start(ov[:, 0:2, ts(j, CH)], och[:, 0:2, :])
                    nc.sync.dma_start(ov[:, 2:4, ts(j, CH)], och[:, 2:4, :])

            # ---- software pipeline over the two batch items
            load_x(0)
            load_x(1)
            alloc_batch(0)
            for j in range(NCH):
                phase1_chunk(0, j)
            alloc_batch(1)
            for j in range(pre_emit):
                phase1_chunk(1, j)
            finalize(0)
            if spread:
                # thin ph2(b0) in the PE-dense middle; the rest interleaves
                # with ph2(b1) in the tail as a second independent stream
                done0 = 0
                for j in range(NCH - pre_emit):
                    if j % 2 == 0:
                        phase2_y(0, done0)
                        phase2_p(0, done0)
                        done0 += 1
                    phase1_chunk(1, j + pre_emit)
                finalize(1)
                for j in range(NCH):
                    if done0 < NCH:
                        phase2_y(0, done0)
                        phase2_p(0, done0)
                        done0 += 1
                    phase2_y(1, j)
                    phase2_p(1, j)
            else:
                for j in range(NCH):
                    if j >= stagger:
                        phase2_y(0, j - stagger)
                    if j + pre_emit < NCH:
                        phase1_chunk(1, j + pre_emit)
                    if j >= stagger:
                        phase2_p(0, j - stagger)
                finalize(1)
                for j in range(NCH - stagger, NCH):
                    phase2_y(0, j)
                    phase2_p(0, j)
            if tail_yfirst:
                for j in range(NCH):
                    phase2_y(1, j)
                for j in range(NCH):
                    phase2_p(1, j)
            elif tail2:
                phase2_y(1, 0)
                phase2_y(1, 1)
                for j in range(2, NCH):
                    phase2_y(1, j)
                    phase2_p(1, j - 2)
                phase2_p(1, NCH - 2)
                phase2_p(1, NCH - 1)
            else:
                phase2_y(1, 0)
                for j in range(1, NCH):
                    phase2_y(1, j)
                    phase2_p(1, j - 1)
                phase2_p(1, NCH - 1)

    nc.compile()
    return nc


def _get_nc():
    if "nc" not in _CACHE:
        _CACHE["nc"] = build_nc()
    return _CACHE["nc"]


def _prep_in_maps(x, wA, wB, wC, wProj, bn_gamma, bn_beta, bn_mean, bn_var,
                  fuse_ab=False):
    bf = ml_dtypes.bfloat16
    scale = (bn_gamma / np.sqrt(bn_var + 1e-5)).astype(np.float32)
    bias = (bn_beta - bn_mean * scale).astype(np.float32)
    wPf = wProj * scale[:, None]

    def wprep(w):
        return np.ascontiguousarray(
            w.T.reshape(KC, 128, HD).transpose(1, 0, 2)).astype(bf)

    wat = wprep(wA)
    wbt = wprep(wB)
    wct = wprep(wC)
    wpt = np.ascontiguousarray(wPf.T.reshape(128, MC, 128)).astype(bf)
    bias2 = np.ascontiguousarray(bias.reshape(MC, 128).T)

    xb = np.ascontiguousarray(x.reshape(B, C, N)).astype(bf)

    in_maps = []
    for i in range(NCORES):
        m = {
            "xs": np.ascontiguousarray(xb[i * BPC:(i + 1) * BPC]),
            "wct": wct, "wpt": wpt, "bias": bias2,
        }
        if fuse_ab:
            m["wab"] = np.ascontiguousarray(
                np.concatenate([wat, wbt], axis=2))
        else:
            m["wat"] = wat
            m["wbt"] = wbt
        in_maps.append(m)
    return in_maps


def kernel(x, wA, wB, wC, wProj, bn_gamma, bn_beta, bn_mean, bn_var):
    x = np.asarray(x, dtype=np.float32)
    wA = np.asarray(wA, dtype=np.float32)
    wB = np.asarray(wB, dtype=np.float32)
    wC = np.asarray(wC, dtype=np.float32)
    wProj = np.asarray(wProj, dtype=np.float32)
    bn_gamma = np.asarray(bn_gamma, dtype=np.float32)
    bn_beta = np.asarray(bn_beta, dtype=np.float32)
    bn_mean = np.asarray(bn_mean, dtype=np.float32)
    bn_var = np.asarray(bn_var, dtype=np.float32)

    in_maps = _prep_in_maps(x, wA, wB, wC, wProj, bn_gamma, bn_beta,
                            bn_mean, bn_var)
    nc = _get_nc()
    res = run_bass_kernel_spmd(nc, in_maps, core_ids=list(range(NCORES)))
    out = np.concatenate([res.results[i]["out"].astype(np.float32)
                          for i in range(NCORES)], axis=0)
    return np.ascontiguousarray(out.reshape(B, C, 64, 64))


# revision 27
# speedup vs baseline: 1.1889x; 1.1889x over previous
"""Trainium2 Bass kernel for the attention-style channel-mixing block
(3x 1x1-conv + double softmax feature gathering + projection + BN).

Data-parallel over batch: B=16 split as 2 batch items per NeuronCore
across 8 cores; weights replicated. No cross-core communication.

Math per batch item (xf = x[b] reshaped [C=512, N=4096]):
    ZA = wA @ xf ; ZB = wB @ xf ; ZC = wC @ xf          (Hd=128 rows)
    A  = softmax(ZA, axis=n) ; Cs = softmax(ZC, axis=n)
    G  = ZB @ A^T                                        [128,128]
    y  = G @ Cs
    out = BN(wProj @ y)

Softmax denominators are folded into the tiny G matrix:
    y = (G * (1/(sumA*sumC))[None,:]) @ expC
so the only full-size nonlinear work is exp() on the scalar engine.
No max-subtraction is needed: |Z| < ~6 so exp() stays in fp32 range.

ZA/ZB are computed directly in transposed layout (n on partitions) so
G needs no separate TensorE transposes; sumA falls out of a
matmul-with-ones that shares G's PSUM accumulation group (single
start=True — a second group start would clear the whole bank's
has_written bits and drop G's first tile).

The two batch items are software-pipelined: phase 2 of batch 0 (y,
projection, store — little PE work) is interleaved instruction-by-
instruction with phase 1 of batch 1 (matmul-dense) so the in-order
TensorE queue never drains.

Compute dtype is bf16 (inputs cast on host, fp32 PSUM accumulation),
output fp32. Measured end-to-end rel err vs the fp32 reference ~4.7e-3.
"""

import contextlib
import sys

if "/opt/trn_rl_repo" not in sys.path:
    sys.path.insert(0, "/opt/trn_rl_repo")

import ml_dtypes
import numpy as np

import concourse.bass as bass
import concourse.mybir as mybir
import concourse.tile as tile
from concourse import bacc
from concourse.bass_utils import run_bass_kernel_spmd
from concourse.masks import make_identity

BF16 = mybir.dt.bfloat16
F32 = mybir.dt.float32

B, C, HD = 16, 512, 128
N = 64 * 64
NCORES = 8
BPC = B // NCORES          # batch items per core
KC = C // 128              # 4 contraction tiles over channels
MC = C // 128              # 4 output-row tiles of the projection
CH = 512                   # free-dim chunk (one PSUM bank of fp32)
NCH = N // CH              # 8 chunks
TPC = CH // 128            # 4 transposed 128-tiles per chunk
XDMA = 4                   # input DMAs per batch item

_CACHE: dict = {}


def ts(i, size):
    return slice(i * size, (i + 1) * size)


def build_nc(defer_g=True, warmup=32, stagger=2, big_bufs=4, p_bufs=3,
             n_act_base=1, fuse_ab=False, z_bufs=3, ab_bufs=2,
             suma_row=False, pre_emit=2, tail2=0, och_bufs=6,
             tail_yfirst=0, ych_bufs=6, spread=0, early_dma=0, xlay=1,
             olay=0):
    nc = bacc.Bacc("TRN2", target_bir_lowering=False, debug=False,
                   num_devices=NCORES)

    if xlay:
        # chunk-contiguous layout: 4 KB contiguous per partition per chunk
        xs = nc.declare_dram_parameter("xs", [BPC, NCH, 128, KC, CH], BF16,
                                       isOutput=False)
    else:
        xs = nc.declare_dram_parameter("xs", [BPC, C, N], BF16,
                                       isOutput=False)
    # All three conv weights are staged as w.T split into K-tiles:
    # [p, kc, h] = w[h, kc*128+p]
    if fuse_ab:
        wab = nc.declare_dram_parameter("wab", [128, KC, 2 * HD], BF16,
                                        isOutput=False)
    else:
        wat = nc.declare_dram_parameter("wat", [128, KC, HD], BF16,
                                        isOutput=False)
        wbt = nc.declare_dram_parameter("wbt", [128, KC, HD], BF16,
                                        isOutput=False)
    wct = nc.declare_dram_parameter("wct", [128, KC, HD], BF16, isOutput=False)
    # Projection (BN scale folded) transposed: [p(h), mc, m] = wProj'[mc*128+m, p]
    wpt = nc.declare_dram_parameter("wpt", [128, MC, 128], BF16, isOutput=False)
    biasd = nc.declare_dram_parameter("bias", [128, MC], F32, isOutput=False)
    if olay:
        # chunk-contiguous output: 2 KB contiguous per partition per half-DMA
        outd = nc.declare_dram_parameter("out", [BPC, NCH, 128, MC, CH],
                                         BF16, isOutput=True)
    else:
        outd = nc.declare_dram_parameter("out", [BPC, C, N], BF16,
                                         isOutput=True)

    with tile.TileContext(nc) as tc:
        with (
            tc.tile_pool(name="const", bufs=1) as cpool,
            tc.tile_pool(name="xin", bufs=2) as xpool,
            tc.tile_pool(name="persist", bufs=2) as ppool,
            tc.tile_pool(name="work", bufs=3) as wpool,
            tc.tile_pool(name="outs", bufs=och_bufs) as opool,
            tc.tile_pool(name="psbig",
                         bufs=(z_bufs if fuse_ab else big_bufs),
                         space="PSUM") as psbig,
            tc.tile_pool(name="psg", bufs=1, space="PSUM") as psg,
            tc.tile_pool(name="psp", bufs=p_bufs, space="PSUM") as psp,
            contextlib.ExitStack() as _ps,
        ):
            psab = (_ps.enter_context(
                tc.tile_pool(name="psab", bufs=ab_bufs, space="PSUM"))
                if fuse_ab else None)
            if warmup:
                warm = cpool.tile([128, 128], BF16, tag="warm")
                nc.gpsimd.memset(warm[:], 0.0)
                psum_warm = psp.tile([128, 128], F32, tag="p", name="warmp")
                for _ in range(warmup):
                    nc.tensor.matmul(psum_warm[:], warm[:], warm[:])

            ident = cpool.tile([128, 128], BF16, tag="ident")
            make_identity(nc, ident[:])
            ones = cpool.tile([128, 1], BF16, tag="ones")
            nc.gpsimd.memset(ones[:], 1.0)
            onesf = cpool.tile([1, 1], F32, tag="onesf")
            nc.gpsimd.memset(onesf[:], 1.0)

            wctt = cpool.tile([128, KC, HD], BF16, tag="wct")
            wptt = cpool.tile([128, MC, 128], BF16, tag="wpt")
            biast = cpool.tile([128, MC], F32, tag="bias")
            nc.scalar.dma_start(wctt[:], wct[:])
            if fuse_ab:
                wabt = cpool.tile([128, KC, 2 * HD], BF16, tag="wab")
                nc.scalar.dma_start(wabt[:], wab[:])
            else:
                watt = cpool.tile([128, KC, HD], BF16, tag="wat")
                wbtt = cpool.tile([128, KC, HD], BF16, tag="wbt")
                nc.scalar.dma_start(watt[:], wat[:])
                nc.scalar.dma_start(wbtt[:], wbt[:])
            nc.scalar.dma_start(wptt[:], wpt[:])
            nc.scalar.dma_start(biast[:], biasd[:])

            # per-batch state, filled in by the emitters below
            st = [dict() for _ in range(BPC)]

            # One PSUM bank holds G+sumA for BOTH batch items ([:, b, :]).
            # Only the literal first G matmul of batch 0 carries start=True;
            # batch 1's first write lands on untouched has_written bits and
            # overwrites per-element, so no second group start is needed
            # (a start would clear the whole bank under batch 0's values).
            gsa_all = psg.tile([128, BPC, 129], F32, tag="g", name="gsa")

            def load_x(b):
                s = st[b]
                s["xt"] = xpool.tile([128, KC, N], BF16, tag="x", name=f"xt{b}")
                if not xlay:
                    xv = xs[b].rearrange("(kc p) n -> p kc n", p=128)
                for h in range(NCH):
                    xvh = xs[b, h] if xlay else xv[:, :, ts(h, CH)]
                    if b == 0 and h < 2:
                        # land the first chunks fast: four parallel engines
                        for kc in range(KC):
                            nc.sync.dma_start(
                                s["xt"][:, kc:kc + 1, ts(h, CH)],
                                xvh[:, kc:kc + 1, :])
                    else:
                        nc.sync.dma_start(s["xt"][:, :, ts(h, CH)], xvh[:])

            def alloc_batch(b):
                s = st[b]
                s["expc"] = ppool.tile([128, N], BF16, tag="expc", name=f"expc{b}")
                s["expat"] = ppool.tile([128, N // 128, 128], BF16, tag="expat", name=f"expat{b}")
                s["bft"] = ppool.tile([128, N // 128, 128], BF16, tag="bft", name=f"bft{b}")
                s["sumcp"] = ppool.tile([128, NCH], F32, tag="sumcp", name=f"sumcp{b}")

            def phase1_chunk(b, j):
                s = st[b]
                xt = s["xt"]

                # ZC = wC @ x  (h on partitions) -> exp + row-sums
                psum_zc = psbig.tile([128, CH], F32, tag="big")
                for kc in range(KC):
                    nc.tensor.matmul(psum_zc[:], wctt[:, kc, :],
                                     xt[:, kc, ts(j, CH)],
                                     start=(kc == 0), stop=(kc == KC - 1))
                nc.scalar.activation(s["expc"][:, ts(j, CH)], psum_zc[:],
                                     mybir.ActivationFunctionType.Exp,
                                     accum_out=s["sumcp"][:, j:j + 1])

                if fuse_ab:
                    # ZA^T and ZB^T in one stream: shared stationary x tile,
                    # rhs = [wA^T | wB^T] (256 cols), halves LDWEIGHTS count
                    for half in range(2):
                        psum_ab = psab.tile([128, 2, 2 * HD], F32, tag="ab",
                                            name=f"ab{b}_{j}_{half}")
                        for tt in range(2):
                            t = 2 * half + tt
                            for kc in range(KC):
                                nc.tensor.matmul(
                                    psum_ab[:, tt, :],
                                    xt[:, kc, ts(j * TPC + t, 128)],
                                    wabt[:, kc, :],
                                    start=(kc == 0), stop=(kc == KC - 1))
                        base = j * TPC + 2 * half
                        nc.scalar.activation(
                            s["expat"][:, base:base + 2, :],
                            psum_ab[:, :, 0:HD],
                            mybir.ActivationFunctionType.Exp)
                        nc.vector.tensor_copy(s["bft"][:, base:base + 2, :],
                                              psum_ab[:, :, HD:2 * HD])
                else:
                    # ZA^T directly (n on partitions): x_tile^T @ wA^T
                    psum_at = psbig.tile([128, TPC, 128], F32, tag="big")
                    for t in range(TPC):
                        for kc in range(KC):
                            nc.tensor.matmul(psum_at[:, t, :],
                                             xt[:, kc, ts(j * TPC + t, 128)],
                                             watt[:, kc, :],
                                             start=(kc == 0),
                                             stop=(kc == KC - 1))
                    nc.scalar.activation(s["expat"][:, ts(j, TPC), :],
                                         psum_at[:],
                                         mybir.ActivationFunctionType.Exp)

                    # ZB^T directly
                    psum_bt = psbig.tile([128, TPC, 128], F32, tag="big")
                    for t in range(TPC):
                        for kc in range(KC):
                            nc.tensor.matmul(psum_bt[:, t, :],
                                             xt[:, kc, ts(j * TPC + t, 128)],
                                             wbtt[:, kc, :],
                                             start=(kc == 0),
                                             stop=(kc == KC - 1))
                    nc.vector.tensor_copy(s["bft"][:, ts(j, TPC), :],
                                          psum_bt[:])

                # G/sumA accumulation for the PREVIOUS chunk: its evacs are
                # long done, so these matmuls never stall the in-order PE
                # queue waiting on DVE/ACT.
                if defer_g:
                    if j > 0:
                        g_accum(b, j - 1)
                else:
                    g_accum(b, j)

            def g_accum(b, j):
                # G += ZB^T.T @ expA^T ; sumA += expA^T.T @ ones
                s = st[b]
                for t in range(TPC):
                    first = (b == 0 and j == 0 and t == 0)
                    last = (b == BPC - 1 and j == NCH - 1 and t == TPC - 1)
                    nc.tensor.matmul(gsa_all[:, b, 0:128],
                                     s["bft"][:, j * TPC + t, :],
                                     s["expat"][:, j * TPC + t, :],
                                     start=first, stop=False,
                                     skip_group_check=True)
                    nc.tensor.matmul(gsa_all[:, b, 128:129],
                                     s["expat"][:, j * TPC + t, :],
                                     ones[:, :1],
                                     start=False, stop=last,
                                     skip_group_check=True)

            def finalize(b):
                s = st[b]
                if defer_g:
                    g_accum(b, NCH - 1)
                sumc = wpool.tile([128, 1], F32, tag="sumc")
                nc.vector.tensor_reduce(sumc[:], s["sumcp"][:],
                                        mybir.AxisListType.X,
                                        mybir.AluOpType.add)
                prod = wpool.tile([128, 1], F32, tag="prod")
                nc.vector.tensor_scalar_mul(prod[:], gsa_all[:, b, 128:129],
                                            sumc[:])
                sinv = wpool.tile([128, 1], F32, tag="sinv")
                nc.vector.reciprocal(sinv[:], prod[:])

                g_sb = wpool.tile([128, 128], BF16, tag="g")
                nc.vector.tensor_copy(g_sb[:], gsa_all[:, b, 0:128])
                psum_gt = psp.tile([128, 128], BF16, tag="p")
                nc.tensor.transpose(psum_gt[:], g_sb[:], ident[:])
                s["gt"] = wpool.tile([128, 128], BF16, tag="gt", name=f"gt{b}")
                nc.scalar.activation(s["gt"][:], psum_gt[:],
                                     mybir.ActivationFunctionType.Copy,
                                     scale=sinv[:, :1])

            def phase2_y(b, j):
                s = st[b]
                psum_y = psbig.tile([128, CH], F32, tag="big")
                nc.tensor.matmul(psum_y[:], s["gt"][:], s["expc"][:, ts(j, CH)])
                s[f"ych{j}"] = wpool.tile([128, CH], BF16, tag="ych", name=f"ych{b}_{j}", bufs=ych_bufs)
                nc.scalar.activation(s[f"ych{j}"][:], psum_y[:],
                                     mybir.ActivationFunctionType.Copy)

            def phase2_p(b, j):
                s = st[b]
                if olay:
                    ov3 = outd[b, j]          # [128, MC, CH]
                else:
                    ov = outd[b].rearrange("(mc p) n -> p mc n", p=128)
                och = opool.tile([128, MC, CH], BF16, tag="och", name=f"och{b}_{j}")
                n_act = n_act_base + (j % 2)  # avg n_act_base+0.5 evacs on ACT
                for mc in range(MC):
                    psum_p = psp.tile([128, CH], F32, tag="p")
                    nc.tensor.matmul(psum_p[:], wptt[:, mc, :], s[f"ych{j}"][:])
                    dst = och[:, mc, :]
                    if mc < n_act:
                        nc.scalar.activation(
                            dst, psum_p[:],
                            mybir.ActivationFunctionType.Identity,
                            bias=biast[:, mc:mc + 1])
                    else:
                        nc.vector.tensor_scalar_add(dst, psum_p[:],
                                                    biast[:, mc:mc + 1])
                    if early_dma and mc == 1 and not (
                            b == BPC - 1 and j >= NCH - 2):
                        # ship the first half as soon as its rows are ready
                        dst0 = ov3[:, 0:2, :] if olay else ov[:, 0:2, ts(j, CH)]
                        nc.sync.dma_start(dst0, och[:, 0:2, :])
                if early_dma and not (b == BPC - 1 and j >= NCH - 2):
                    dst1 = ov3[:, 2:4, :] if olay else ov[:, 2:4, ts(j, CH)]
                    nc.sync.dma_start(dst1, och[:, 2:4, :])
                elif b == BPC - 1 and j >= NCH - 2:
                    # end-of-kernel drain: max DMA-engine parallelism
                    for mc in range(MC):
                        dstm = (ov3[:, mc:mc + 1, :] if olay
                                else ov[:, mc:mc + 1, ts(j, CH)])
                        nc.sync.dma_start(dstm, och[:, mc:mc + 1, :])
                elif not early_dma:
                    if olay:
                        nc.sync.dma_start(ov3[:, 0:2, :], och[:, 0:2, :])
                        nc.sync.dma_start(ov3[:, 2:4, :], och[:, 2:4, :])
                    else:
                        nc.sync.dma_start(ov[:, 0:2, ts(j, CH)],
                                          och[:, 0:2, :])
                        nc.sync.dma_start(ov[:, 2:4, ts(j, CH)],
                                          och[:, 2:4, :])

            # ---- software pipeline over the two batch items
            load_x(0)
            load_x(1)
            alloc_batch(0)
            for j in range(NCH):
                phase1_chunk(0, j)
            alloc_batch(1)
            for j in range(pre_emit):
                phase1_chunk(1, j)
            finalize(0)
            if spread:
                # thin ph2(b0) in the PE-dense middle; the rest interleaves
                # with ph2(b1) in the tail as a second independent stream
                done0 = 0
                for j in range(NCH - pre_emit):
                    if j % 2 == 0:
                        phase2_y(0, done0)
                        phase2_p(0, done0)
                        done0 += 1
                    phase1_chunk(1, j + pre_emit)
                finalize(1)
                for j in range(NCH):
                    if done0 < NCH:
                        phase2_y(0, done0)
                        phase2_p(0, done0)
                        done0 += 1
                    phase2_y(1, j)
                    phase2_p(1, j)
            else:
                for j in range(NCH):
                    if j >= stagger:
                        phase2_y(0, j - stagger)
                    if j + pre_emit < NCH:
                        phase1_chunk(1, j + pre_emit)
                    if j >= stagger:
                        phase2_p(0, j - stagger)
                finalize(1)
                for j in range(NCH - stagger, NCH):
                    phase2_y(0, j)
                    phase2_p(0, j)
            if tail_yfirst:
                for j in range(NCH):
                    phase2_y(1, j)
                for j in range(NCH):
                    phase2_p(1, j)
            elif tail2:
                phase2_y(1, 0)
                phase2_y(1, 1)
                for j in range(2, NCH):
                    phase2_y(1, j)
                    phase2_p(1, j - 2)
                phase2_p(1, NCH - 2)
                phase2_p(1, NCH - 1)
            else:
                phase2_y(1, 0)
                for j in range(1, NCH):
                    phase2_y(1, j)
                    phase2_p(1, j - 1)
                phase2_p(1, NCH - 1)

    nc.compile()
    return nc


def _get_nc():
    if "nc" not in _CACHE:
        _CACHE["nc"] = build_nc()
    return _CACHE["nc"]


def _prep_in_maps(x, wA, wB, wC, wProj, bn_gamma, bn_beta, bn_mean, bn_var,
                  fuse_ab=False, xlay=1):
    bf = ml_dtypes.bfloat16
    scale = (bn_gamma / np.sqrt(bn_var + 1e-5)).astype(np.float32)
    bias = (bn_beta - bn_mean * scale).astype(np.float32)
    wPf = wProj * scale[:, None]

    def wprep(w):
        return np.ascontiguousarray(
            w.T.reshape(KC, 128, HD).transpose(1, 0, 2)).astype(bf)

    wat = wprep(wA)
    wbt = wprep(wB)
    wct = wprep(wC)
    wpt = np.ascontiguousarray(wPf.T.reshape(128, MC, 128)).astype(bf)
    bias2 = np.ascontiguousarray(bias.reshape(MC, 128).T)

    xb = np.ascontiguousarray(x.reshape(B, C, N)).astype(bf)
    if xlay:
        # [b, j, p, kc, n]: per-chunk DMA reads 4 KB contiguous/partition
        xb = np.ascontiguousarray(
            xb.reshape(B, KC, 128, NCH, CH).transpose(0, 3, 2, 1, 4))

    in_maps = []
    for i in range(NCORES):
        m = {
            "xs": np.ascontiguousarray(xb[i * BPC:(i + 1) * BPC]),
            "wct": wct, "wpt": wpt, "bias": bias2,
        }
        if fuse_ab:
            m["wab"] = np.ascontiguousarray(
                np.concatenate([wat, wbt], axis=2))
        else:
            m["wat"] = wat
            m["wbt"] = wbt
        in_maps.append(m)
    return in_maps


def _unshard(outs, olay=0):
    arr = np.concatenate([np.asarray(o).astype(np.float32) for o in outs],
                         axis=0)
    if olay:
        # [B, NCH, 128, MC, CH] -> [B, MC, 128, NCH, CH] -> [B, C, N]
        arr = np.ascontiguousarray(arr.transpose(0, 3, 2, 1, 4)).reshape(
            B, C, N)
    return np.ascontiguousarray(arr.reshape(B, C, 64, 64))


def kernel(x, wA, wB, wC, wProj, bn_gamma, bn_beta, bn_mean, bn_var):
    x = np.asarray(x, dtype=np.float32)
    wA = np.asarray(wA, dtype=np.float32)
    wB = np.asarray(wB, dtype=np.float32)
    wC = np.asarray(wC, dtype=np.float32)
    wProj = np.asarray(wProj, dtype=np.float32)
    bn_gamma = np.asarray(bn_gamma, dtype=np.float32)
    bn_beta = np.asarray(bn_beta, dtype=np.float32)
    bn_mean = np.asarray(bn_mean, dtype=np.float32)
    bn_var = np.asarray(bn_var, dtype=np.float32)

    in_maps = _prep_in_maps(x, wA, wB, wC, wProj, bn_gamma, bn_beta,
                            bn_mean, bn_var)
    nc = _get_nc()
    res = run_bass_kernel_spmd(nc, in_maps, core_ids=list(range(NCORES)))
    return _unshard([res.results[i]["out"] for i in range(NCORES)],
                    olay=0)
